# revision 1
# baseline (speedup 1.0000x reference)
"""LSTM encoder with EOS-freeze for Trainium2, data-parallel over batch on 8 cores.

Strategy
--------
Inputs are one-hot, so x @ Wi is a row-gather of Wi done with indirect DMA on
device. The recurrent h @ Wh runs on the tensor engine with Wh as 64 fp16
[128,128] stationary tiles (FWL) and h.T chunks as the [128,16] moving operand,
producing z transposed: PSUM [128 partitions = z-feature % 128, 16*tile + b].
Gates are reordered (i, f, o, g) host-side and the g block pre-scaled by 2 so a
single sigmoid over all 2048 gate columns yields tanh(g) = 2*sigmoid(2g) - 1.

The EOS freeze is handled without any per-step masking: sequences are
independent, so the kernel runs the unmasked recurrence and streams per-step
(c, h) snapshots to DRAM; the frozen value for sequence b is the snapshot at
its first-EOS step, selected during unshard.
"""

import numpy as np

try:
    import concourse  # noqa: F401
except ImportError:
    import sys

    sys.path.insert(0, "/opt/trn_rl_repo")

from contextlib import ExitStack

import concourse.bass as bass
import concourse.tile as tile
from concourse import bacc
from concourse import mybir
from concourse.bass import ds
from concourse.bass_utils import run_bass_kernel_spmd

dt = mybir.dt
Alu = mybir.AluOpType
Act = mybir.ActivationFunctionType

EOS_ID = 1
HID = 512
BATCH, SEQ, VOCAB = 128, 256, 1024
GATES = 4 * HID  # 2048
NCORES = 8
BLOC = BATCH // NCORES  # 16 sequences per core
NT = GATES // 128  # 16 feature tiles of z
NK = HID // 128  # 4 contraction chunks
BODY = 16  # steps per For_i iteration

# Collect profiling info when True (set by test.py; adds trace overhead).
TRACE = False
LAST_RESULTS = None  # BassKernelResults of the last run, for test.py

_PROGRAM = None


def _build_program(seq=SEQ, body=BODY):
    nc = bacc.Bacc("TRN2", debug=False, detect_race_conditions=False)

    wi = nc.declare_dram_parameter("wi", [VOCAB, GATES], dt.float16, isOutput=False)
    ident = nc.declare_dram_parameter("ident", [BLOC, BLOC], dt.float16, isOutput=False)
    wh = nc.declare_dram_parameter("wh", [128, NK * NT * 128], dt.float16, isOutput=False)
    tok = nc.declare_dram_parameter("tok", [BLOC, seq + body], dt.int32, isOutput=False)
    c_traj = nc.declare_dram_parameter("c_traj", [seq * 128, 64], dt.float32, isOutput=True)
    h_traj = nc.declare_dram_parameter("h_traj", [seq * 128, 64], dt.float16, isOutput=True)

    with tile.TileContext(nc) as tc, ExitStack() as ctx:
        pool = lambda name, bufs, **kw: ctx.enter_context(
            tc.tile_pool(name=name, bufs=bufs, **kw)
        )
        whp = pool("whp", 1)
        tokp = pool("tokp", 1)
        stp = pool("stp", 1)
        hp = pool("hp", 1)
        cp = pool("cp", 1)
        zp_pool = pool("zp", 2, space="PSUM")
        sp = pool("sp", 2)
        gp = pool("gp", 2)
        ap_ = pool("ap", 2)
        bp = pool("bp", 2)
        s2p = pool("s2p", 2)
        tp = pool("tp", 2)

        wh_sb = whp.tile([128, NK * NT * 128], dt.float16, name="wh_sb")
        nc.sync.dma_start(out=wh_sb[:], in_=wh[:, :])
        tok_cur = tokp.tile([BLOC, body], dt.int32, name="tok_cur")
        nc.sync.dma_start(out=tok_cur[:], in_=tok[:, 0:body])
        id_sb = tokp.tile([BLOC, BLOC], dt.float16, name="id_sb")
        nc.sync.dma_start(out=id_sb[:], in_=ident[:, :])

        ST = [stp.tile([BLOC, GATES], dt.float16, name=f"st{s}", tag=f"st{s}") for s in range(body)]
        H = [hp.tile([128, 64], dt.float16, name=f"h{s}", tag=f"h{s}") for s in range(body)]
        C = [cp.tile([128, 64], dt.float32, name=f"c{s}", tag=f"c{s}") for s in range(body)]

        nc.gpsimd.memset(H[body - 1][:], 0.0)
        nc.gpsimd.memset(C[body - 1][:], 0.0)
        for s in range(body):
            # init shadow coverage; real values come from the indirect gathers
            nc.gpsimd.memset(ST[s][:], 0.0)

        def gather_xp(s):
            # Gather BLOC wi rows (one per sequence) for one timestep into
            # ST[s][b, :] — row-per-partition, the DGE-supported shape.
            # tok_cur always holds the token column for the block being
            # prefetched, so the offset AP stays static.
            nc.gpsimd.indirect_dma_start(
                out=ST[s][:],
                out_offset=None,
                in_=wi[:, :],
                in_offset=bass.IndirectOffsetOnAxis(ap=tok_cur[:, s : s + 1], axis=0),
            )

        for s in range(body):
            gather_xp(s)

        def step(iv, s):
            hprev = H[(s - 1) % body]
            cprev = C[(s - 1) % body]
            zps = zp_pool.tile([128, 256], dt.float32, name="zps", tag="zpsum")
            # x@Wi enters PSUM via PE transpose of the gathered rows: these
            # matmuls need no h, so they overlap the previous step's tail.
            for t in range(NT):
                # start=True on the first matmul clears the bank's has_written
                # bits; every other matmul joins the same accumulation group.
                nc.tensor.matmul(
                    out=zps[:, 16 * t : 16 * t + 16],
                    lhsT=ST[s][:, 128 * t : 128 * t + 128],
                    rhs=id_sb[:],
                    start=(t == 0),
                    stop=False,
                )
            for k in range(NK):
                for t in range(NT):
                    nc.tensor.matmul(
                        out=zps[:, 16 * t : 16 * t + 16],
                        lhsT=wh_sb[:, (k * NT + t) * 128 : (k * NT + t) * 128 + 128],
                        rhs=hprev[:, 16 * k : 16 * k + 16],
                        start=False,
                        stop=(k == NK - 1 and t == NT - 1),
                    )
            S = sp.tile([128, 192], dt.float32, name="S", tag="S")
            nc.scalar.activation(out=S[:], in_=zps[:, 0:192], func=Act.Sigmoid)
            TG = gp.tile([128, 64], dt.float16, name="TG", tag="TG")
            nc.scalar.activation(out=TG[:], in_=zps[:, 192:256], func=Act.Tanh)
            A = ap_.tile([128, 64], dt.float32, name="A", tag="A")
            nc.vector.tensor_tensor(out=A[:], in0=S[:, 0:64], in1=TG[:], op=Alu.mult)
            B = bp.tile([128, 64], dt.float32, name="B", tag="B")
            nc.vector.tensor_tensor(out=B[:], in0=S[:, 64:128], in1=cprev[:], op=Alu.mult)
            cs = C[s]
            nc.vector.tensor_tensor(out=cs[:], in0=A[:], in1=B[:], op=Alu.add)
            T = tp.tile([128, 64], dt.float16, name="T", tag="T")
            nc.scalar.activation(out=T[:], in_=cs[:], func=Act.Tanh)
            hs = H[s]
            nc.vector.tensor_tensor(out=hs[:], in0=S[:, 128:192], in1=T[:], op=Alu.mult)

            nc.sync.dma_start(out=c_traj[ds((iv + s) * 128, 128), :], in_=cs[:])
            nc.sync.dma_start(out=h_traj[ds((iv + s) * 128, 128), :], in_=hs[:])
            # Prefetch this slot's xp for the next block (the token table is
            # padded so the final block reads harmless extra rows).
            gather_xp(s)

        with tc.For_i(0, seq, body, hint_engines=(mybir.EngineType.PE,), staggered_reset=True) as iv:
            # Stage the NEXT block's token columns; in-loop gathers prefetch
            # for block i+1 while this block computes.
            nc.sync.dma_start(out=tok_cur[:], in_=tok[:, ds(iv + body, body)])
            for s in range(body):
                step(iv, s)

    nc.finalize()
    return nc


def _get_program():
    global _PROGRAM
    if _PROGRAM is None:
        _PROGRAM = _build_program()
    return _PROGRAM


def _prep_host(inputs, Wi, Wh, b):
    tokens = np.argmax(inputs, axis=-1).astype(np.int32)  # [B, T]
    eos = inputs[:, :, EOS_ID] > 0.5
    any_eos = eos.any(axis=1)
    t_star = np.where(any_eos, eos.argmax(axis=1), SEQ - 1).astype(np.int64)

    # Gate reorder (i, f, o, g): one contiguous sigmoid over i,f,o and one
    # tanh over g (both live in the same ACT table set).
    perm = np.concatenate(
        [np.arange(0, 512), np.arange(512, 1024), np.arange(1536, 2048), np.arange(1024, 1536)]
    )
    Wi_re = (Wi.astype(np.float32) + b.astype(np.float32)[None, :])[:, perm]
    Wh_re = Wh.astype(np.float32)[:, perm]

    # wi rows stay in z-feature order (gate-permuted only); wh tile k*16+t
    # holds Wh_re[128k:128k+128, 128t:128t+128], stored partition-major.
    Wi_dev = np.ascontiguousarray(Wi_re).astype(np.float16)
    # Partition-major: wh[kr, (k*NT+t)*128 + p] = Wh_re[128k+kr, 128t+p]
    Wh_dev = np.ascontiguousarray(
        Wh_re.reshape(NK, 128, NT, 128).transpose(1, 0, 2, 3).reshape(128, NK * NT * 128)
    ).astype(np.float16)
    return tokens, t_star, Wi_dev, Wh_dev


def kernel(inputs, Wi, Wh, b):
    global LAST_RESULTS
    inputs = np.asarray(inputs)
    Wi = np.asarray(Wi)
    Wh = np.asarray(Wh)
    b = np.asarray(b)

    tokens, t_star, Wi_dev, Wh_dev = _prep_host(inputs, Wi, Wh, b)

    in_maps = []
    for n in range(NCORES):
        tokc = tokens[BLOC * n : BLOC * (n + 1)]
        tok_pad = np.concatenate([tokc, np.zeros((BLOC, BODY), np.int32)], axis=1)
        in_maps.append(
            {
                "wi": Wi_dev,
                "wh": Wh_dev,
                "tok": np.ascontiguousarray(tok_pad),
                "ident": np.eye(BLOC, dtype=np.float16),
            }
        )

    nc = _get_program()
    res = run_bass_kernel_spmd(nc, in_maps, list(range(NCORES)), trace=TRACE)
    LAST_RESULTS = res

    c_out = np.zeros((BATCH, HID), np.float32)
    h_out = np.zeros((BATCH, HID), np.float32)
    for n in range(NCORES):
        ct = res.results[n]["c_traj"].reshape(SEQ, 128, 64)
        ht = res.results[n]["h_traj"].reshape(SEQ, 128, 64).astype(np.float32)
        for bl in range(BLOC):
            g = BLOC * n + bl
            t = int(t_star[g])
            c_out[g] = ct[t][:, bl::BLOC].T.reshape(HID)
            h_out[g] = ht[t][:, bl::BLOC].T.reshape(HID)
    return (c_out, h_out)

